# revision 9
# baseline (speedup 1.0000x reference)
"""CrossRMSD Trainium2 kernel.

Computes pairwise RMSD between S mobile and T target structures:
  R(s,t) = Xm_s^T Xt_t  (3x3 cross-covariance, contraction over atoms on PE)
  lambda_max of the 4x4 quaternion key matrix F(R) via Newton iteration on the
  QCP quartic characteristic polynomial  x^4 + C2 x^2 + C1 x + C0
  (Theobald 2005), started from the upper bound x0 = sqrt(3*q), q = sum R_ij^2
  (valid since tr F = 0, tr F^2 = 4q  =>  lmax <= sqrt(3/4 * tr F^2)).
  RMSD = sqrt(relu((|Xm|^2 + |Xt|^2 - 2*lmax) / (A + eps)))

Sharding: S axis split across 8 cores (data parallel); X_target replicated.
"""

import sys
import types

sys.path.insert(0, "/opt/trn_rl_repo")

import numpy as np

import bass_rust
import concourse.bass as bass
import concourse.mybir as mybir
from concourse import tile
from concourse.bass_utils import run_bass_kernel_spmd

F32 = mybir.dt.float32
ALU = mybir.AluOpType
ACTF = mybir.ActivationFunctionType

N_CORES = 8
S_FULL, A_ATOMS, T_FULL = 2048, 128, 2048
S_LOC = S_FULL // N_CORES  # 256
FD = 512  # free-dim chunk (one PSUM bank of f32)
NEWTON_ITERS = 2
EPS = 1e-5
XM_SCALE = 1.0 / 32.0  # host-side scale on Xm so q, C0 stay in fp16 range
W_DAMP = 0.09  # constant damping for the Newton step y <- y - w*p(y)


# ---------------------------------------------------------------- infra patches
def _install_axon_patches():
    """Two environment fixes:
    1. Split the TileContext end-drain sem waits (this walrus build's TPB_CTRL
       encodes at most one sync wait per instruction).
    2. Provide antenv.axon_hooks so trace=True works under axon (optional).
    """

    def patched_drain(self, tick_clock, wait_clock):
        from concourse.tile import ScopedClock

        probe = self.nc.sync.nop(nofuse=True)
        wait_clock.add_sem_waits(
            probe.ins, ScopedClock({None: tick_clock.global_clock})
        )
        si = probe.ins.sync_info
        waits = list(si.on_wait or []) if si is not None else []
        if si is not None:
            probe.ins.sync_info = bass_rust.SyncInfo(on_wait=waits[:1], on_update=[])
        rest = waits[1:]
        while rest:
            chunk, rest = rest[:1], rest[1:]
            n = self.nc.sync.nop(nofuse=True)
            n.ins.sync_info = bass_rust.SyncInfo(on_wait=chunk, on_update=[])
        self.nc.sync.drain()
        self.nc.all_engine_barrier()
        assert self.sems is not None
        popped = self.nc._tile_sem_poison_stack.pop()
        assert popped is self._sem_poison
        self.nc.clear_and_free_semaphores(list(self.sems.allocated().values()))
        self.nc.all_engine_barrier()

    tile.TileContext._drain_and_barrier = patched_drain

    if "antenv.axon_hooks" not in sys.modules:
        import contextlib
        import ctypes

        def _mk_hook():
            try:
                lib = ctypes.CDLL("/opt/axon/libaxon_pjrt.so")
            except OSError:
                return None
            if not hasattr(lib, "axon_start_nrt_profile"):
                return None
            lib.axon_start_nrt_profile.argtypes = [
                ctypes.POINTER(ctypes.c_int64),
                ctypes.c_size_t,
            ]
            lib.axon_start_nrt_profile.restype = ctypes.c_int64
            lib.axon_stop_nrt_profile.argtypes = [ctypes.c_char_p]
            lib.axon_stop_nrt_profile.restype = ctypes.c_int64

            @contextlib.contextmanager
            def _hook(output_dir, device_ids):
                import jax

                jax.devices()
                if device_ids:
                    ids = (ctypes.c_int64 * len(device_ids))(*device_ids)
                    rc = lib.axon_start_nrt_profile(ids, len(device_ids))
                else:
                    rc = lib.axon_start_nrt_profile(None, 0)
                if rc != 0:
                    raise RuntimeError(f"axon_start_nrt_profile rc={rc}")
                try:
                    yield
                finally:
                    n = lib.axon_stop_nrt_profile(str(output_dir).encode())
                    if n < 0:
                        raise RuntimeError(f"axon_stop_nrt_profile rc={n}")

            return _hook

        hook = _mk_hook()
        mod = types.ModuleType("antenv.axon_hooks")
        mod.get_axon_ntff_profile_hook = lambda: hook
        mod.set_axon_ntff_profile_hook = lambda h: None
        sys.modules["antenv.axon_hooks"] = mod


_install_axon_patches()


def _split_multi_waits(nc):
    """This walrus build encodes at most one sync wait per instruction; hoist
    extra waits onto same-engine NoOps placed immediately before."""
    for fn in nc.m.functions:
        for bb in fn.blocks:
            out = []
            for inst in bb.instructions:
                si = inst.sync_info
                waits = list(si.on_wait or []) if si is not None else []
                if len(waits) > 1:
                    for wchunk in waits[:-1]:
                        nop = mybir.InstNoOp(
                            name=nc.get_next_instruction_name(), ins=[], outs=[]
                        )
                        nop.engine = inst.engine
                        nop.sync_info = bass_rust.SyncInfo(
                            on_wait=[wchunk], on_update=[]
                        )
                        nc.register_instruction(nop)
                        out.append(nop)
                    inst.sync_info = bass_rust.SyncInfo(
                        on_wait=[waits[-1]],
                        on_update=list(si.on_update or []),
                    )
                out.append(inst)
            bb.instructions[:] = out


# ---------------------------------------------------------------- device kernel
BF16 = mybir.dt.bfloat16
F16 = mybir.dt.float16

# lmax/sqrt(q) ~ quadratic fit in (sd, s2) = (detR/q^1.5, C0/q^2); start point
# for the fixed-weight Newton iteration (lstsq on the actual feature
# distribution, rms resid 0.022).
B_QUAD = [1.429768871731997, 2.372311619434571, -0.22358362125748854,
          1.6180680207138314, -7.9808587338341646, -0.09846030115589527]


class Slots:
    """Workspace allocator: fresh pool tile per logical value, tag-recycled
    so SBUF footprint stays bounded at n slots x bufs."""

    def __init__(self, pool, n, shape, dtype, prefix):
        self.pool = pool
        self.shape = list(shape)
        self.dtype = dtype
        self.prefix = prefix
        self.free = list(range(n))[::-1]
        self.named = {}
        self.peak = 0

    def new(self, name):
        j = self.free.pop()
        t = self.pool.tile(
            self.shape, self.dtype, name=f"{self.prefix}{j}_{name}",
            tag=f"{self.prefix}{j}",
        )
        self.named[name] = (j, t)
        self.peak = max(self.peak, len(self.named))
        return t

    def __getitem__(self, name):
        return self.named[name][1]

    def drop(self, *names):
        for nm in names:
            j, _ = self.named.pop(nm)
            self.free.append(j)


DEBUG_TAPS = {}  # name -> dram tensor; filled by build_nc(debug=True)


def _tap(nc, name, ap, sb, tn):
    if DEBUG_TAPS and (sb, tn) == (0, 0) and name in DEBUG_TAPS:
        nc.sync.dma_start(out=DEBUG_TAPS[name][:], in_=ap)


def _emit_tile(nc, tc, pools, xm_s, xt_s, gm_s, gt_s, out_dram, sb, tn):
    """One [128, FD] output tile, all-fp16 elementwise pipeline.

    R rows land in PSUM (Xm pre-scaled by 1/32 so q, C0 stay in fp16 range);
    q, C0 = 2 tr(M^2) - q^2, detR computed in fp16 (Vector 2x mode);
    lmax via fixed-weight Newton y <- y - w*p(y) on the normalized quartic
    y^4 - 2 y^2 + s1 y + s2, with w = 1/clamp(p'(y0)) via reciprocal_approx,
    started from a quadratic fit y0(sd, s2).
    """
    psum_row, psum_n, wide, nb, nf, outp = pools
    V, G, SC = nc.vector, nc.gpsimd, nc.scalar

    ssl = slice(sb * 128, (sb + 1) * 128)
    tsl = slice(tn * FD, (tn + 1) * FD)

    W = Slots(wide, 8, [128, 3, FD], F16, "W")
    B = Slots(nb, 14, [128, FD], F16, "B")
    F = Slots(nf, 8, [128, FD], F32, "F")

    def wtt(dst, x, y, op, eng=V):
        eng.tensor_tensor(out=dst, in0=x, in1=y, op=op)

    # --- PE: R rows (R_kj = Xm_k . Xt_j over atoms) + N = Gm+Gt ------------
    prs = []
    for k in range(3):
        pr = psum_row.tile([128, 3, FD], F32, name=f"pr{k}", tag="pr")
        for j in range(3):
            nc.tensor.matmul(
                pr[:, j, :], xm_s[:, k, ssl], xt_s[:, j, tsl],
                start=True, stop=True,
            )
        prs.append(pr)
    npl = psum_n.tile([128, FD], F32, name="npsum", tag="npsum")
    nc.tensor.matmul(npl[:], gm_s[:, ssl], gt_s[:, tsl], start=True, stop=True)

    # --- ACT: downcast rows to fp16 ----------------------------------------
    for k in range(3):
        rb = W.new(f"row{k}")
        SC.copy(rb[:], prs[k][:])

    def RP(k, j):  # fp16 R_kj plane
        return W[f"row{k}"][:, j, :]

    # --- M = R^T R diagonal -> q (V wide fp16; G narrow folds) -------------
    for k in range(3):
        sq = W.new(f"sq{k}")
        wtt(sq[:], W[f"row{k}"][:], W[f"row{k}"][:], ALU.mult, V)
    md1 = W.new("md1")
    wtt(md1[:], W["sq0"][:], W["sq1"][:], ALU.add, V)
    W.drop("sq0", "sq1")
    mdiag = W.new("mdiag")
    wtt(mdiag[:], md1[:], W["sq2"][:], ALU.add, V)
    W.drop("sq2", "md1")

    q1 = B.new("q1")
    wtt(q1[:], mdiag[:, 0, :], mdiag[:, 1, :], ALU.add, G)
    qf = F.new("qf")
    wtt(qf[:], q1[:], mdiag[:, 2, :], ALU.add, G)
    B.drop("q1")

    sqMd = W.new("sqMd")
    SC.activation(sqMd[:], mdiag[:], ACTF.Square)
    W.drop("mdiag")
    z1 = B.new("z1")
    wtt(z1[:], sqMd[:, 0, :], sqMd[:, 1, :], ALU.add, G)
    z2 = B.new("z2")
    wtt(z2[:], z1[:], sqMd[:, 2, :], ALU.add, G)
    B.drop("z1")
    W.drop("sqMd")

    # --- M off-diagonal -> z4 (V wide fp16) --------------------------------
    for k in range(3):
        p = W.new(f"prod{k}")
        rb = W[f"row{k}"]
        wtt(p[:, 0:2, :], rb[:, 0:2, :], rb[:, 1:3, :], ALU.mult, V)
        wtt(p[:, 2, :], rb[:, 2, :], rb[:, 0, :], ALU.mult, V)
    mo1 = W.new("mo1")
    wtt(mo1[:], W["prod0"][:], W["prod1"][:], ALU.add, G)
    W.drop("prod0", "prod1")
    moff = W.new("moff")
    wtt(moff[:], mo1[:], W["prod2"][:], ALU.add, V)
    W.drop("prod2", "mo1")
    sqMo = W.new("sqMo")
    SC.activation(sqMo[:], moff[:], ACTF.Square)
    W.drop("moff")
    z3 = B.new("z3")
    wtt(z3[:], sqMo[:, 0, :], sqMo[:, 1, :], ALU.add, G)
    z4 = B.new("z4")
    wtt(z4[:], z3[:], sqMo[:, 2, :], ALU.add, G)
    B.drop("z3")
    W.drop("sqMo")

    # --- detR (fp16): det = a(ei-fh) - b(di-fg) + c(dh-eg) -----------------
    d_, e_, f_ = RP(1, 0), RP(1, 1), RP(1, 2)
    g_, h_, i_ = RP(2, 0), RP(2, 1), RP(2, 2)
    detA = W.new("detA")
    wtt(detA[:, 0, :], e_, i_, ALU.mult, G)
    wtt(detA[:, 1, :], d_, i_, ALU.mult, G)
    wtt(detA[:, 2, :], d_, h_, ALU.mult, G)
    detB = W.new("detB")
    wtt(detB[:, 0, :], f_, h_, ALU.mult, V)
    wtt(detB[:, 1, :], f_, g_, ALU.mult, V)
    wtt(detB[:, 2, :], e_, g_, ALU.mult, V)
    detC = W.new("detC")
    wtt(detC[:], detA[:], detB[:], ALU.subtract, V)
    W.drop("detA", "detB")
    tp = W.new("tp")
    wtt(tp[:], W["row0"][:], detC[:], ALU.mult, V)
    W.drop("detC", "row0", "row1", "row2")
    dt1 = B.new("dt1")
    wtt(dt1[:], tp[:, 0, :], tp[:, 1, :], ALU.subtract, G)
    detb = B.new("det")
    wtt(detb[:], dt1[:], tp[:, 2, :], ALU.add, G)
    B.drop("dt1")
    W.drop("tp")

    # --- C0 = 2*(2*z4 + z2) - q^2 (fp16, q^2 in fp32) ----------------------
    qq = F.new("qq")
    SC.activation(qq[:], qf[:], ACTF.Square)
    u16 = B.new("u16")
    V.scalar_tensor_tensor(out=u16[:], in0=z4[:], scalar=2.0, in1=z2[:],
                           op0=ALU.mult, op1=ALU.add)
    B.drop("z2", "z4")
    C0b = B.new("C0")
    V.scalar_tensor_tensor(out=C0b[:], in0=u16[:], scalar=2.0, in1=qq[:],
                           op0=ALU.mult, op1=ALU.subtract)
    B.drop("u16")
    F.drop("qq")

    # --- features: sd = det/q^1.5, s2 = C0/q^2 (one reciprocal) ------------
    sqq = F.new("sqq")
    SC.activation(sqq[:], qf[:], ACTF.Sqrt)
    F.drop("qf")
    rsq = F.new("rsq")
    V.reciprocal(rsq[:], sqq[:])  # q^-0.5
    rq = F.new("rq")
    wtt(rq[:], rsq[:], rsq[:], ALU.mult, V)  # q^-1
    t1f = F.new("t1f")
    wtt(t1f[:], detb[:], rq[:], ALU.mult, V)
    B.drop("det")
    sdb = B.new("sd")
    wtt(sdb[:], t1f[:], rsq[:], ALU.mult, V)
    F.drop("t1f", "rsq")
    t2f = F.new("t2f")
    wtt(t2f[:], C0b[:], rq[:], ALU.mult, V)
    B.drop("C0")
    s2b = B.new("s2")
    wtt(s2b[:], t2f[:], rq[:], ALU.mult, V)
    F.drop("t2f", "rq")

    cb = B.new("c")
    V.tensor_scalar(out=cb[:], in0=sdb[:], scalar1=-8.0, scalar2=-2.0,
                    op0=ALU.mult, op1=ALU.add)  # c = s1 - 2 = -8*sd - 2

    # --- quadratic fit y0 = B0+B1*sd+B2*s2+B3*sd*s2+B4*sd^2+B5*s2^2 --------
    Q = B_QUAD
    sds2 = B.new("sds2")
    wtt(sds2[:], sdb[:], s2b[:], ALU.mult, V)
    sd2 = B.new("sd2")
    SC.activation(sd2[:], sdb[:], ACTF.Square)
    s22 = B.new("s22")
    SC.activation(s22[:], s2b[:], ACTF.Square)
    g1 = B.new("g1")
    V.tensor_scalar(out=g1[:], in0=sdb[:], scalar1=Q[1], scalar2=Q[0],
                    op0=ALU.mult, op1=ALU.add)
    g2 = B.new("g2")
    V.scalar_tensor_tensor(out=g2[:], in0=s2b[:], scalar=Q[2], in1=g1[:],
                           op0=ALU.mult, op1=ALU.add)
    B.drop("g1")
    g3 = B.new("g3")
    V.scalar_tensor_tensor(out=g3[:], in0=sds2[:], scalar=Q[3], in1=g2[:],
                           op0=ALU.mult, op1=ALU.add)
    B.drop("g2", "sds2")
    g4 = B.new("g4")
    V.scalar_tensor_tensor(out=g4[:], in0=sd2[:], scalar=Q[4], in1=g3[:],
                           op0=ALU.mult, op1=ALU.add)
    B.drop("g3", "sd2")
    y = B.new("y")
    V.scalar_tensor_tensor(out=y[:], in0=s22[:], scalar=Q[5], in1=g4[:],
                           op0=ALU.mult, op1=ALU.add)
    B.drop("g4", "s22")

    # --- damped Newton: y <- y - w*((y^2 + (s1-2))*y + s2) -----------------
    for _it in range(NEWTON_ITERS):
        y2 = B.new("y2")
        SC.activation(y2[:], y[:], ACTF.Square)
        bb = B.new("b")
        wtt(bb[:], y2[:], cb[:], ALU.add, V)
        B.drop("y2")
        dd = B.new("d")
        wtt(dd[:], bb[:], y[:], ALU.mult, V)
        B.drop("b")
        num = B.new("num")
        wtt(num[:], dd[:], s2b[:], ALU.add, V)
        B.drop("d")
        yn = B.new("yn")
        V.scalar_tensor_tensor(out=yn[:], in0=num[:], scalar=-W_DAMP, in1=y[:],
                               op0=ALU.mult, op1=ALU.add)
        B.drop("y", "num")
        B.named["y"] = B.named.pop("yn")
        y = B["y"]
    B.drop("s2", "c")

    # --- output: rmsd = sqrt(relu((N - 64*lam)/(A+eps))) -------------------
    lam = F.new("lam")
    wtt(lam[:], y[:], sqq[:], ALU.mult, V)
    B.drop("y")
    F.drop("sqq")
    f2 = F.new("f2")
    V.scalar_tensor_tensor(out=f2[:], in0=lam[:], scalar=-64.0, in1=npl[:],
                           op0=ALU.mult, op1=ALU.add)
    F.drop("lam")
    rl = F.new("rl")
    V.tensor_scalar_max(rl[:], f2[:], 0.0)
    F.drop("f2")
    ot = outp.tile([128, FD], F32, name="out_t", tag="out")
    SC.activation(ot[:], rl[:], ACTF.Sqrt, scale=1.0 / (A_ATOMS + EPS))
    F.drop("rl")
    nc.sync.dma_start(out=out_dram[ssl, tsl], in_=ot[:])


def build_nc(debug_taps=()):
    nc = bass.Bass()
    DEBUG_TAPS.clear()
    for nm in debug_taps:
        dt_ = BF16 if nm in ("q", "C0", "det", "sd", "s2") else F32
        DEBUG_TAPS[nm] = nc.declare_dram_parameter(
            f"dbg_{nm}", [128, FD], dt_, isOutput=True)
    xm = nc.declare_dram_parameter("xm", [A_ATOMS, 3, S_LOC], F32, isOutput=False)
    xt = nc.declare_dram_parameter("xt", [A_ATOMS, 3, T_FULL], F32, isOutput=False)
    gm = nc.declare_dram_parameter("gm", [2, S_LOC], F32, isOutput=False)
    gt = nc.declare_dram_parameter("gt", [2, T_FULL], F32, isOutput=False)
    out = nc.declare_dram_parameter("out", [S_LOC, T_FULL], F32, isOutput=True)

    with tile.TileContext(nc) as tc, nc.allow_low_precision(
        reason="bf16 coefficient pipeline; validated vs reference"
    ):
        with (
            tc.tile_pool(name="const", bufs=1) as const,
            tc.tile_pool(name="psum_row", bufs=2, space="PSUM") as psum_row,
            tc.tile_pool(name="psum_n", bufs=2, space="PSUM") as psum_n,
            tc.tile_pool(name="wide", bufs=2) as wide,
            tc.tile_pool(name="nb", bufs=2) as nb,
            tc.tile_pool(name="nf", bufs=2) as nf,
            tc.tile_pool(name="outp", bufs=3) as outp,
        ):
            xm_s = const.tile([A_ATOMS, 3, S_LOC], F32)
            xt_s = const.tile([A_ATOMS, 3, T_FULL], F32)
            gm_s = const.tile([2, S_LOC], F32)
            gt_s = const.tile([2, T_FULL], F32)
            nc.sync.dma_start(out=xm_s[:], in_=xm[:])
            for k in range(8):  # split the big replicated load across queues
                sl = slice(k * (T_FULL // 8), (k + 1) * (T_FULL // 8))
                nc.sync.dma_start(out=xt_s[:, :, sl], in_=xt[:, :, sl])
            nc.sync.dma_start(out=gm_s[:], in_=gm[:])
            nc.sync.dma_start(out=gt_s[:], in_=gt[:])

            pools = (psum_row, psum_n, wide, nb, nf, outp)
            for sb in range(S_LOC // 128):
                for tn in range(T_FULL // FD):
                    _emit_tile(nc, tc, pools, xm_s, xt_s, gm_s, gt_s, out, sb, tn)
    return nc


_NC_CACHE = None


def _get_nc():
    global _NC_CACHE
    if _NC_CACHE is None:
        nc = build_nc()
        _split_multi_waits(nc)
        _NC_CACHE = nc
    return _NC_CACHE


# ---------------------------------------------------------------- host wrapper
def kernel(X_mobile: np.ndarray, X_target: np.ndarray, **_ignored) -> np.ndarray:
    Xm = np.ascontiguousarray(X_mobile, dtype=np.float32)
    Xt = np.ascontiguousarray(X_target, dtype=np.float32)
    S, A, _ = Xm.shape
    T = Xt.shape[0]
    assert (S, A, T) == (S_FULL, A_ATOMS, T_FULL), (S, A, T)

    Xmc = Xm - Xm.mean(axis=1, keepdims=True)
    Xtc = Xt - Xt.mean(axis=1, keepdims=True)
    Gm = (Xmc * Xmc).sum(axis=(1, 2))
    Gt = (Xtc * Xtc).sum(axis=(1, 2))

    xt_r = np.ascontiguousarray(Xtc.transpose(1, 2, 0))  # [A, 3, T]
    gt2 = np.ascontiguousarray(
        np.stack([np.ones(T, np.float32), Gt.astype(np.float32)])
    )

    in_maps = []
    for c in range(N_CORES):
        sl = slice(c * S_LOC, (c + 1) * S_LOC)
        xm_l = np.ascontiguousarray(
            (Xmc[sl] * XM_SCALE).transpose(1, 2, 0)
        )  # [A, 3, S_loc]
        gm2 = np.ascontiguousarray(
            np.stack([Gm[sl].astype(np.float32), np.ones(S_LOC, np.float32)])
        )
        in_maps.append({"xm": xm_l, "xt": xt_r, "gm": gm2, "gt": gt2})

    nc = _get_nc()
    res = run_bass_kernel_spmd(nc, in_maps, list(range(N_CORES)))
    return np.concatenate([res.results[c]["out"] for c in range(N_CORES)], axis=0)


def run_traced(X_mobile, X_target):
    """test.py helper: same as kernel() but with NTFF tracing enabled."""
    Xm = np.ascontiguousarray(X_mobile, dtype=np.float32)
    Xt = np.ascontiguousarray(X_target, dtype=np.float32)
    Xmc = Xm - Xm.mean(axis=1, keepdims=True)
    Xtc = Xt - Xt.mean(axis=1, keepdims=True)
    Gm = (Xmc * Xmc).sum(axis=(1, 2))
    Gt = (Xtc * Xtc).sum(axis=(1, 2))
    xt_r = np.ascontiguousarray(Xtc.transpose(1, 2, 0))
    gt2 = np.ascontiguousarray(
        np.stack([np.ones(T_FULL, np.float32), Gt.astype(np.float32)])
    )
    in_maps = []
    for c in range(N_CORES):
        sl = slice(c * S_LOC, (c + 1) * S_LOC)
        xm_l = np.ascontiguousarray((Xmc[sl] * XM_SCALE).transpose(1, 2, 0))
        gm2 = np.ascontiguousarray(
            np.stack([Gm[sl].astype(np.float32), np.ones(S_LOC, np.float32)])
        )
        in_maps.append({"xm": xm_l, "xt": xt_r, "gm": gm2, "gt": gt2})
    nc = _get_nc()
    res = run_bass_kernel_spmd(nc, in_maps, list(range(N_CORES)), trace=True)
    out = np.concatenate([res.results[c]["out"] for c in range(N_CORES)], axis=0)
    return out, res



# revision 20
# speedup vs baseline: 1.0439x; 1.0439x over previous
"""CrossRMSD Trainium2 kernel.

Computes pairwise RMSD between S mobile and T target structures:
  R(s,t) = Xm_s^T Xt_t  (3x3 cross-covariance, contraction over atoms on PE)
  lambda_max of the 4x4 quaternion key matrix F(R) via Newton iteration on the
  QCP quartic characteristic polynomial  x^4 + C2 x^2 + C1 x + C0
  (Theobald 2005), started from the upper bound x0 = sqrt(3*q), q = sum R_ij^2
  (valid since tr F = 0, tr F^2 = 4q  =>  lmax <= sqrt(3/4 * tr F^2)).
  RMSD = sqrt(relu((|Xm|^2 + |Xt|^2 - 2*lmax) / (A + eps)))

Sharding: S axis split across 8 cores (data parallel); X_target replicated.
"""

import sys
import types

sys.path.insert(0, "/opt/trn_rl_repo")

import numpy as np

import bass_rust
import concourse.bass as bass
import concourse.mybir as mybir
from concourse import tile
from concourse.bass_utils import run_bass_kernel_spmd

F32 = mybir.dt.float32
ALU = mybir.AluOpType
ACTF = mybir.ActivationFunctionType

N_CORES = 8
S_FULL, A_ATOMS, T_FULL = 2048, 128, 2048
S_LOC = S_FULL // N_CORES  # 256
FD = 512  # free-dim chunk (one PSUM bank of f32)
NEWTON_ITERS = 2
EPS = 1e-5
XM_SCALE = 1.0 / 32.0  # host-side scale on Xm so q, C0 stay in fp16 range
W_DAMP = 0.09  # constant damping for the Newton step y <- y - w*p(y)


# ---------------------------------------------------------------- infra patches
def _install_axon_patches():
    """Two environment fixes:
    1. Split the TileContext end-drain sem waits (this walrus build's TPB_CTRL
       encodes at most one sync wait per instruction).
    2. Provide antenv.axon_hooks so trace=True works under axon (optional).
    """

    def patched_drain(self, tick_clock, wait_clock):
        from concourse.tile import ScopedClock

        probe = self.nc.sync.nop(nofuse=True)
        wait_clock.add_sem_waits(
            probe.ins, ScopedClock({None: tick_clock.global_clock})
        )
        si = probe.ins.sync_info
        waits = list(si.on_wait or []) if si is not None else []
        if si is not None:
            probe.ins.sync_info = bass_rust.SyncInfo(on_wait=waits[:1], on_update=[])
        rest = waits[1:]
        while rest:
            chunk, rest = rest[:1], rest[1:]
            n = self.nc.sync.nop(nofuse=True)
            n.ins.sync_info = bass_rust.SyncInfo(on_wait=chunk, on_update=[])
        self.nc.sync.drain()
        self.nc.all_engine_barrier()
        assert self.sems is not None
        popped = self.nc._tile_sem_poison_stack.pop()
        assert popped is self._sem_poison
        self.nc.clear_and_free_semaphores(list(self.sems.allocated().values()))
        self.nc.all_engine_barrier()

    tile.TileContext._drain_and_barrier = patched_drain

    if "antenv.axon_hooks" not in sys.modules:
        import contextlib
        import ctypes

        def _mk_hook():
            try:
                lib = ctypes.CDLL("/opt/axon/libaxon_pjrt.so")
            except OSError:
                return None
            if not hasattr(lib, "axon_start_nrt_profile"):
                return None
            lib.axon_start_nrt_profile.argtypes = [
                ctypes.POINTER(ctypes.c_int64),
                ctypes.c_size_t,
            ]
            lib.axon_start_nrt_profile.restype = ctypes.c_int64
            lib.axon_stop_nrt_profile.argtypes = [ctypes.c_char_p]
            lib.axon_stop_nrt_profile.restype = ctypes.c_int64

            @contextlib.contextmanager
            def _hook(output_dir, device_ids):
                import jax

                jax.devices()
                if device_ids:
                    ids = (ctypes.c_int64 * len(device_ids))(*device_ids)
                    rc = lib.axon_start_nrt_profile(ids, len(device_ids))
                else:
                    rc = lib.axon_start_nrt_profile(None, 0)
                if rc != 0:
                    raise RuntimeError(f"axon_start_nrt_profile rc={rc}")
                try:
                    yield
                finally:
                    n = lib.axon_stop_nrt_profile(str(output_dir).encode())
                    if n < 0:
                        raise RuntimeError(f"axon_stop_nrt_profile rc={n}")

            return _hook

        hook = _mk_hook()
        mod = types.ModuleType("antenv.axon_hooks")
        mod.get_axon_ntff_profile_hook = lambda: hook
        mod.set_axon_ntff_profile_hook = lambda h: None
        sys.modules["antenv.axon_hooks"] = mod


_install_axon_patches()


def _split_multi_waits(nc):
    """This walrus build encodes at most one sync wait per instruction; hoist
    extra waits onto same-engine NoOps placed immediately before."""
    for fn in nc.m.functions:
        for bb in fn.blocks:
            out = []
            for inst in bb.instructions:
                si = inst.sync_info
                waits = list(si.on_wait or []) if si is not None else []
                if len(waits) > 1:
                    for wchunk in waits[:-1]:
                        nop = mybir.InstNoOp(
                            name=nc.get_next_instruction_name(), ins=[], outs=[]
                        )
                        nop.engine = inst.engine
                        nop.sync_info = bass_rust.SyncInfo(
                            on_wait=[wchunk], on_update=[]
                        )
                        nc.register_instruction(nop)
                        out.append(nop)
                    inst.sync_info = bass_rust.SyncInfo(
                        on_wait=[waits[-1]],
                        on_update=list(si.on_update or []),
                    )
                out.append(inst)
            bb.instructions[:] = out


# ---------------------------------------------------------------- device kernel
BF16 = mybir.dt.bfloat16
F16 = mybir.dt.float16

# lmax/sqrt(q) ~ quadratic fit in (sd, s2) = (detR/q^1.5, C0/q^2); start point
# for the fixed-weight Newton iteration (lstsq on the actual feature
# distribution, rms resid 0.022).
B_QUAD = [1.429768871731997, 2.372311619434571, -0.22358362125748854,
          1.6180680207138314, -7.9808587338341646, -0.09846030115589527]


class Slots:
    """Workspace allocator: fresh pool tile per logical value, tag-recycled
    so SBUF footprint stays bounded at n slots x bufs."""

    def __init__(self, pool, n, shape, dtype, prefix):
        self.pool = pool
        self.shape = list(shape)
        self.dtype = dtype
        self.prefix = prefix
        self.free = list(range(n))[::-1]
        self.named = {}
        self.peak = 0

    def new(self, name):
        j = self.free.pop()
        t = self.pool.tile(
            self.shape, self.dtype, name=f"{self.prefix}{j}_{name}",
            tag=f"{self.prefix}{j}",
        )
        self.named[name] = (j, t)
        self.peak = max(self.peak, len(self.named))
        return t

    def __getitem__(self, name):
        return self.named[name][1]

    def drop(self, *names):
        for nm in names:
            j, _ = self.named.pop(nm)
            self.free.append(j)


DEBUG_TAPS = {}  # name -> dram tensor; filled by build_nc(debug=True)


def _tap(nc, name, ap, sb, tn):
    if DEBUG_TAPS and (sb, tn) == (0, 0) and name in DEBUG_TAPS:
        nc.sync.dma_start(out=DEBUG_TAPS[name][:], in_=ap)


def _emit_tile(nc, tc, pools, xm_s, xt_s, gm_s, gt_s, out_dram, sb, tn):
    """One [128, FD] output tile, all-fp16 elementwise pipeline.

    R rows land in PSUM (Xm pre-scaled by 1/32 so q, C0 stay in fp16 range);
    q, C0 = 2 tr(M^2) - q^2, detR computed in fp16 (Vector 2x mode);
    lmax via fixed-weight Newton y <- y - w*p(y) on the normalized quartic
    y^4 - 2 y^2 + s1 y + s2, with w = 1/clamp(p'(y0)) via reciprocal_approx,
    started from a quadratic fit y0(sd, s2).
    """
    psum_row, psum_n, wide, nb, nf, outp = pools
    V, G, SC = nc.vector, nc.gpsimd, nc.scalar

    ssl = slice(sb * 128, (sb + 1) * 128)
    tsl = slice(tn * FD, (tn + 1) * FD)

    W = Slots(wide, 8, [128, 3, FD], F16, "W")
    B = Slots(nb, 14, [128, FD], F16, "B")
    F = Slots(nf, 8, [128, FD], F32, "F")

    def wtt(dst, x, y, op, eng=V):
        eng.tensor_tensor(out=dst, in0=x, in1=y, op=op)

    # --- PE: R rows (R_kj = Xm_k . Xt_j over atoms) + N = Gm+Gt ------------
    prs = []
    for k in range(3):
        pr = psum_row.tile([128, 3, FD], F32, name=f"pr{k}", tag="pr")
        for j in range(3):
            nc.tensor.matmul(
                pr[:, j, :], xm_s[:, k, ssl], xt_s[:, j, tsl],
                start=True, stop=True,
            )
        prs.append(pr)
    npl = psum_n.tile([128, FD], F32, name="npsum", tag="npsum")
    nc.tensor.matmul(npl[:], gm_s[:, ssl], gt_s[:, tsl], start=True, stop=True)

    # --- ACT: downcast rows to fp16 ----------------------------------------
    for k in range(3):
        rb = W.new(f"row{k}")
        SC.copy(rb[:], prs[k][:])

    def RP(k, j):  # fp16 R_kj plane
        return W[f"row{k}"][:, j, :]

    # --- M = R^T R diagonal -> q (V wide fp16; G narrow folds) -------------
    for k in range(3):
        sq = W.new(f"sq{k}")
        wtt(sq[:], W[f"row{k}"][:], W[f"row{k}"][:], ALU.mult, V)
    md1 = W.new("md1")
    wtt(md1[:], W["sq0"][:], W["sq1"][:], ALU.add, V)
    W.drop("sq0", "sq1")
    mdiag = W.new("mdiag")
    wtt(mdiag[:], md1[:], W["sq2"][:], ALU.add, V)
    W.drop("sq2", "md1")

    q1 = B.new("q1")
    wtt(q1[:], mdiag[:, 0, :], mdiag[:, 1, :], ALU.add, V)
    qf = F.new("qf")
    wtt(qf[:], q1[:], mdiag[:, 2, :], ALU.add, G)
    B.drop("q1")

    sqMd = W.new("sqMd")
    wtt(sqMd[:], mdiag[:], mdiag[:], ALU.mult, V)
    W.drop("mdiag")
    z1 = B.new("z1")
    wtt(z1[:], sqMd[:, 0, :], sqMd[:, 1, :], ALU.add, G)
    z2 = B.new("z2")
    wtt(z2[:], z1[:], sqMd[:, 2, :], ALU.add, G)
    B.drop("z1")
    W.drop("sqMd")

    # --- M off-diagonal -> z4 (V wide fp16) --------------------------------
    for k in range(3):
        p = W.new(f"prod{k}")
        rb = W[f"row{k}"]
        wtt(p[:, 0:2, :], rb[:, 0:2, :], rb[:, 1:3, :], ALU.mult, V)
        wtt(p[:, 2, :], rb[:, 2, :], rb[:, 0, :], ALU.mult, V)
    mo1 = W.new("mo1")
    wtt(mo1[:], W["prod0"][:], W["prod1"][:], ALU.add, V)
    W.drop("prod0", "prod1")
    moff = W.new("moff")
    wtt(moff[:], mo1[:], W["prod2"][:], ALU.add, V)
    W.drop("prod2", "mo1")
    sqMo = W.new("sqMo")
    wtt(sqMo[:], moff[:], moff[:], ALU.mult, V)
    W.drop("moff")
    z3 = B.new("z3")
    wtt(z3[:], sqMo[:, 0, :], sqMo[:, 1, :], ALU.add, G)
    z4 = B.new("z4")
    wtt(z4[:], z3[:], sqMo[:, 2, :], ALU.add, G)
    B.drop("z3")
    W.drop("sqMo")

    # --- detR (fp16): det = a(ei-fh) - b(di-fg) + c(dh-eg) -----------------
    d_, e_, f_ = RP(1, 0), RP(1, 1), RP(1, 2)
    g_, h_, i_ = RP(2, 0), RP(2, 1), RP(2, 2)
    detA = W.new("detA")
    wtt(detA[:, 0, :], e_, i_, ALU.mult, G)
    wtt(detA[:, 1, :], d_, i_, ALU.mult, G)
    wtt(detA[:, 2, :], d_, h_, ALU.mult, G)
    detB = W.new("detB")
    wtt(detB[:, 0, :], f_, h_, ALU.mult, V)
    wtt(detB[:, 1, :], f_, g_, ALU.mult, V)
    wtt(detB[:, 2, :], e_, g_, ALU.mult, V)
    detC = W.new("detC")
    wtt(detC[:], detA[:], detB[:], ALU.subtract, V)
    W.drop("detA", "detB")
    tp = W.new("tp")
    wtt(tp[:], W["row0"][:], detC[:], ALU.mult, V)
    W.drop("detC", "row0", "row1", "row2")
    dt1 = B.new("dt1")
    wtt(dt1[:], tp[:, 0, :], tp[:, 1, :], ALU.subtract, G)
    detb = B.new("det")
    wtt(detb[:], dt1[:], tp[:, 2, :], ALU.add, G)
    B.drop("dt1")
    W.drop("tp")

    # --- C0 = 2*(2*z4 + z2) - q^2 (fp16, q^2 in fp32) ----------------------
    qq = F.new("qq")
    wtt(qq[:], qf[:], qf[:], ALU.mult, G)
    u16 = B.new("u16")
    V.scalar_tensor_tensor(out=u16[:], in0=z4[:], scalar=2.0, in1=z2[:],
                           op0=ALU.mult, op1=ALU.add)
    B.drop("z2", "z4")
    C0b = B.new("C0")
    V.scalar_tensor_tensor(out=C0b[:], in0=u16[:], scalar=2.0, in1=qq[:],
                           op0=ALU.mult, op1=ALU.subtract)
    B.drop("u16")
    F.drop("qq")

    # --- features: sd = det*q^-1.5, s2 = C0*q^-2 via Ln/Exp (Scalar eng) ---
    sqq = F.new("sqq")
    SC.activation(sqq[:], qf[:], ACTF.Sqrt)
    lnq = F.new("lnq")
    SC.activation(lnq[:], qf[:], ACTF.Ln)
    F.drop("qf")
    rq15 = F.new("rq15")
    SC.activation(rq15[:], lnq[:], ACTF.Exp, scale=-1.5)
    rq2 = F.new("rq2")
    SC.activation(rq2[:], lnq[:], ACTF.Exp, scale=-2.0)
    F.drop("lnq")
    sdb = B.new("sd")
    wtt(sdb[:], detb[:], rq15[:], ALU.mult, G)
    B.drop("det")
    F.drop("rq15")
    s2b = B.new("s2")
    wtt(s2b[:], C0b[:], rq2[:], ALU.mult, G)
    B.drop("C0")
    F.drop("rq2")

    cb = B.new("c")
    V.tensor_scalar(out=cb[:], in0=sdb[:], scalar1=-8.0, scalar2=-2.0,
                    op0=ALU.mult, op1=ALU.add)  # c = s1 - 2 = -8*sd - 2

    # --- quadratic fit y0 = B0+B1*sd+B2*s2+B3*sd*s2+B4*sd^2+B5*s2^2 --------
    Q = B_QUAD
    sds2 = B.new("sds2")
    wtt(sds2[:], sdb[:], s2b[:], ALU.mult, V)
    sd2 = B.new("sd2")
    SC.activation(sd2[:], sdb[:], ACTF.Square)
    s22 = B.new("s22")
    SC.activation(s22[:], s2b[:], ACTF.Square)
    g1 = B.new("g1")
    V.tensor_scalar(out=g1[:], in0=sdb[:], scalar1=Q[1], scalar2=Q[0],
                    op0=ALU.mult, op1=ALU.add)
    g2 = B.new("g2")
    V.scalar_tensor_tensor(out=g2[:], in0=s2b[:], scalar=Q[2], in1=g1[:],
                           op0=ALU.mult, op1=ALU.add)
    B.drop("g1")
    g3 = B.new("g3")
    V.scalar_tensor_tensor(out=g3[:], in0=sds2[:], scalar=Q[3], in1=g2[:],
                           op0=ALU.mult, op1=ALU.add)
    B.drop("g2", "sds2")
    g4 = B.new("g4")
    V.scalar_tensor_tensor(out=g4[:], in0=sd2[:], scalar=Q[4], in1=g3[:],
                           op0=ALU.mult, op1=ALU.add)
    B.drop("g3", "sd2")
    y = B.new("y")
    V.scalar_tensor_tensor(out=y[:], in0=s22[:], scalar=Q[5], in1=g4[:],
                           op0=ALU.mult, op1=ALU.add)
    B.drop("g4", "s22")

    # --- damped Newton: y <- y - w*((y^2 + (s1-2))*y + s2) -----------------
    for _it in range(NEWTON_ITERS):
        y2 = B.new("y2")
        SC.activation(y2[:], y[:], ACTF.Square)
        bb = B.new("b")
        wtt(bb[:], y2[:], cb[:], ALU.add, V)
        B.drop("y2")
        dd = B.new("d")
        wtt(dd[:], bb[:], y[:], ALU.mult, V)
        B.drop("b")
        num = B.new("num")
        wtt(num[:], dd[:], s2b[:], ALU.add, V)
        B.drop("d")
        yn = B.new("yn")
        V.scalar_tensor_tensor(out=yn[:], in0=num[:], scalar=-W_DAMP, in1=y[:],
                               op0=ALU.mult, op1=ALU.add)
        B.drop("y", "num")
        B.named["y"] = B.named.pop("yn")
        y = B["y"]
    B.drop("s2", "c")

    # --- output: rmsd = sqrt(relu((N - 64*lam)/(A+eps))) -------------------
    lam = F.new("lam")
    wtt(lam[:], y[:], sqq[:], ALU.mult, G)
    B.drop("y")
    F.drop("sqq")
    f2 = F.new("f2")
    V.scalar_tensor_tensor(out=f2[:], in0=lam[:], scalar=-64.0, in1=npl[:],
                           op0=ALU.mult, op1=ALU.add)
    F.drop("lam")
    rl = F.new("rl")
    V.tensor_scalar_max(rl[:], f2[:], 0.0)
    F.drop("f2")
    ot = outp.tile([128, FD], F32, name="out_t", tag="out")
    SC.activation(ot[:], rl[:], ACTF.Sqrt, scale=1.0 / (A_ATOMS + EPS))
    F.drop("rl")
    nc.sync.dma_start(out=out_dram[ssl, tsl], in_=ot[:])


def build_nc(debug_taps=()):
    nc = bass.Bass()
    DEBUG_TAPS.clear()
    for nm in debug_taps:
        dt_ = BF16 if nm in ("q", "C0", "det", "sd", "s2") else F32
        DEBUG_TAPS[nm] = nc.declare_dram_parameter(
            f"dbg_{nm}", [128, FD], dt_, isOutput=True)
    xm = nc.declare_dram_parameter("xm", [A_ATOMS, 3, S_LOC], F32, isOutput=False)
    xt = nc.declare_dram_parameter("xt", [A_ATOMS, 3, T_FULL], F32, isOutput=False)
    gm = nc.declare_dram_parameter("gm", [2, S_LOC], F32, isOutput=False)
    gt = nc.declare_dram_parameter("gt", [2, T_FULL], F32, isOutput=False)
    out = nc.declare_dram_parameter("out", [S_LOC, T_FULL], F32, isOutput=True)

    with tile.TileContext(nc) as tc, nc.allow_low_precision(
        reason="bf16 coefficient pipeline; validated vs reference"
    ):
        with (
            tc.tile_pool(name="const", bufs=1) as const,
            tc.tile_pool(name="psum_row", bufs=2, space="PSUM") as psum_row,
            tc.tile_pool(name="psum_n", bufs=2, space="PSUM") as psum_n,
            tc.tile_pool(name="wide", bufs=2) as wide,
            tc.tile_pool(name="nb", bufs=3) as nb,
            tc.tile_pool(name="nf", bufs=3) as nf,
            tc.tile_pool(name="outp", bufs=3) as outp,
        ):
            xm_s = const.tile([A_ATOMS, 3, S_LOC], F32)
            xt_s = const.tile([A_ATOMS, 3, T_FULL], F32)
            gm_s = const.tile([2, S_LOC], F32)
            gt_s = const.tile([2, T_FULL], F32)
            nc.sync.dma_start(out=xm_s[:], in_=xm[:])
            for k in range(8):  # split the big replicated load across queues
                sl = slice(k * (T_FULL // 8), (k + 1) * (T_FULL // 8))
                nc.sync.dma_start(out=xt_s[:, :, sl], in_=xt[:, :, sl])
            nc.sync.dma_start(out=gm_s[:], in_=gm[:])
            nc.sync.dma_start(out=gt_s[:], in_=gt[:])

            pools = (psum_row, psum_n, wide, nb, nf, outp)
            for sb in range(S_LOC // 128):
                for tn in range(T_FULL // FD):
                    _emit_tile(nc, tc, pools, xm_s, xt_s, gm_s, gt_s, out, sb, tn)
    return nc


_NC_CACHE = None


def _get_nc():
    global _NC_CACHE
    if _NC_CACHE is None:
        nc = build_nc()
        _split_multi_waits(nc)
        _NC_CACHE = nc
    return _NC_CACHE


# ---------------------------------------------------------------- host wrapper
def kernel(X_mobile: np.ndarray, X_target: np.ndarray, **_ignored) -> np.ndarray:
    Xm = np.ascontiguousarray(X_mobile, dtype=np.float32)
    Xt = np.ascontiguousarray(X_target, dtype=np.float32)
    S, A, _ = Xm.shape
    T = Xt.shape[0]
    assert (S, A, T) == (S_FULL, A_ATOMS, T_FULL), (S, A, T)

    Xmc = Xm - Xm.mean(axis=1, keepdims=True)
    Xtc = Xt - Xt.mean(axis=1, keepdims=True)
    Gm = (Xmc * Xmc).sum(axis=(1, 2))
    Gt = (Xtc * Xtc).sum(axis=(1, 2))

    xt_r = np.ascontiguousarray(Xtc.transpose(1, 2, 0))  # [A, 3, T]
    gt2 = np.ascontiguousarray(
        np.stack([np.ones(T, np.float32), Gt.astype(np.float32)])
    )

    in_maps = []
    for c in range(N_CORES):
        sl = slice(c * S_LOC, (c + 1) * S_LOC)
        xm_l = np.ascontiguousarray(
            (Xmc[sl] * XM_SCALE).transpose(1, 2, 0)
        )  # [A, 3, S_loc]
        gm2 = np.ascontiguousarray(
            np.stack([Gm[sl].astype(np.float32), np.ones(S_LOC, np.float32)])
        )
        in_maps.append({"xm": xm_l, "xt": xt_r, "gm": gm2, "gt": gt2})

    nc = _get_nc()
    res = run_bass_kernel_spmd(nc, in_maps, list(range(N_CORES)))
    return np.concatenate([res.results[c]["out"] for c in range(N_CORES)], axis=0)


def run_traced(X_mobile, X_target):
    """test.py helper: same as kernel() but with NTFF tracing enabled."""
    Xm = np.ascontiguousarray(X_mobile, dtype=np.float32)
    Xt = np.ascontiguousarray(X_target, dtype=np.float32)
    Xmc = Xm - Xm.mean(axis=1, keepdims=True)
    Xtc = Xt - Xt.mean(axis=1, keepdims=True)
    Gm = (Xmc * Xmc).sum(axis=(1, 2))
    Gt = (Xtc * Xtc).sum(axis=(1, 2))
    xt_r = np.ascontiguousarray(Xtc.transpose(1, 2, 0))
    gt2 = np.ascontiguousarray(
        np.stack([np.ones(T_FULL, np.float32), Gt.astype(np.float32)])
    )
    in_maps = []
    for c in range(N_CORES):
        sl = slice(c * S_LOC, (c + 1) * S_LOC)
        xm_l = np.ascontiguousarray((Xmc[sl] * XM_SCALE).transpose(1, 2, 0))
        gm2 = np.ascontiguousarray(
            np.stack([Gm[sl].astype(np.float32), np.ones(S_LOC, np.float32)])
        )
        in_maps.append({"xm": xm_l, "xt": xt_r, "gm": gm2, "gt": gt2})
    nc = _get_nc()
    res = run_bass_kernel_spmd(nc, in_maps, list(range(N_CORES)), trace=True)
    out = np.concatenate([res.results[c]["out"] for c in range(N_CORES)], axis=0)
    return out, res



# revision 24
# speedup vs baseline: 1.0639x; 1.0192x over previous
"""CrossRMSD Trainium2 kernel.

Computes pairwise RMSD between S mobile and T target structures:
  R(s,t) = Xm_s^T Xt_t  (3x3 cross-covariance, contraction over atoms on PE)
  lambda_max of the 4x4 quaternion key matrix F(R) via Newton iteration on the
  QCP quartic characteristic polynomial  x^4 + C2 x^2 + C1 x + C0
  (Theobald 2005), started from the upper bound x0 = sqrt(3*q), q = sum R_ij^2
  (valid since tr F = 0, tr F^2 = 4q  =>  lmax <= sqrt(3/4 * tr F^2)).
  RMSD = sqrt(relu((|Xm|^2 + |Xt|^2 - 2*lmax) / (A + eps)))

Sharding: S axis split across 8 cores (data parallel); X_target replicated.
"""

import sys
import types

sys.path.insert(0, "/opt/trn_rl_repo")

import numpy as np

import bass_rust
import concourse.bass as bass
import concourse.mybir as mybir
from concourse import tile
from concourse.bass_utils import run_bass_kernel_spmd

F32 = mybir.dt.float32
ALU = mybir.AluOpType
ACTF = mybir.ActivationFunctionType

N_CORES = 8
S_FULL, A_ATOMS, T_FULL = 2048, 128, 2048
S_LOC = S_FULL // N_CORES  # 256
FD = 512  # free-dim chunk (one PSUM bank of f32)
NEWTON_ITERS = 2
EPS = 1e-5
XM_SCALE = 1.0 / 32.0  # host-side scale on Xm so q, C0 stay in fp16 range
W_DAMP = 0.09  # constant damping for the Newton step y <- y - w*p(y)


# ---------------------------------------------------------------- infra patches
def _install_axon_patches():
    """Two environment fixes:
    1. Split the TileContext end-drain sem waits (this walrus build's TPB_CTRL
       encodes at most one sync wait per instruction).
    2. Provide antenv.axon_hooks so trace=True works under axon (optional).
    """

    def patched_drain(self, tick_clock, wait_clock):
        from concourse.tile import ScopedClock

        probe = self.nc.sync.nop(nofuse=True)
        wait_clock.add_sem_waits(
            probe.ins, ScopedClock({None: tick_clock.global_clock})
        )
        si = probe.ins.sync_info
        waits = list(si.on_wait or []) if si is not None else []
        if si is not None:
            probe.ins.sync_info = bass_rust.SyncInfo(on_wait=waits[:1], on_update=[])
        rest = waits[1:]
        while rest:
            chunk, rest = rest[:1], rest[1:]
            n = self.nc.sync.nop(nofuse=True)
            n.ins.sync_info = bass_rust.SyncInfo(on_wait=chunk, on_update=[])
        self.nc.sync.drain()
        self.nc.all_engine_barrier()
        assert self.sems is not None
        popped = self.nc._tile_sem_poison_stack.pop()
        assert popped is self._sem_poison
        self.nc.clear_and_free_semaphores(list(self.sems.allocated().values()))
        self.nc.all_engine_barrier()

    tile.TileContext._drain_and_barrier = patched_drain

    if "antenv.axon_hooks" not in sys.modules:
        import contextlib
        import ctypes

        def _mk_hook():
            try:
                lib = ctypes.CDLL("/opt/axon/libaxon_pjrt.so")
            except OSError:
                return None
            if not hasattr(lib, "axon_start_nrt_profile"):
                return None
            lib.axon_start_nrt_profile.argtypes = [
                ctypes.POINTER(ctypes.c_int64),
                ctypes.c_size_t,
            ]
            lib.axon_start_nrt_profile.restype = ctypes.c_int64
            lib.axon_stop_nrt_profile.argtypes = [ctypes.c_char_p]
            lib.axon_stop_nrt_profile.restype = ctypes.c_int64

            @contextlib.contextmanager
            def _hook(output_dir, device_ids):
                import jax

                jax.devices()
                if device_ids:
                    ids = (ctypes.c_int64 * len(device_ids))(*device_ids)
                    rc = lib.axon_start_nrt_profile(ids, len(device_ids))
                else:
                    rc = lib.axon_start_nrt_profile(None, 0)
                if rc != 0:
                    raise RuntimeError(f"axon_start_nrt_profile rc={rc}")
                try:
                    yield
                finally:
                    n = lib.axon_stop_nrt_profile(str(output_dir).encode())
                    if n < 0:
                        raise RuntimeError(f"axon_stop_nrt_profile rc={n}")

            return _hook

        hook = _mk_hook()
        mod = types.ModuleType("antenv.axon_hooks")
        mod.get_axon_ntff_profile_hook = lambda: hook
        mod.set_axon_ntff_profile_hook = lambda h: None
        sys.modules["antenv.axon_hooks"] = mod


_install_axon_patches()


def _split_multi_waits(nc):
    """This walrus build encodes at most one sync wait per instruction; hoist
    extra waits onto same-engine NoOps placed immediately before."""
    for fn in nc.m.functions:
        for bb in fn.blocks:
            out = []
            for inst in bb.instructions:
                si = inst.sync_info
                waits = list(si.on_wait or []) if si is not None else []
                if len(waits) > 1:
                    for wchunk in waits[:-1]:
                        nop = mybir.InstNoOp(
                            name=nc.get_next_instruction_name(), ins=[], outs=[]
                        )
                        nop.engine = inst.engine
                        nop.sync_info = bass_rust.SyncInfo(
                            on_wait=[wchunk], on_update=[]
                        )
                        nc.register_instruction(nop)
                        out.append(nop)
                    inst.sync_info = bass_rust.SyncInfo(
                        on_wait=[waits[-1]],
                        on_update=list(si.on_update or []),
                    )
                out.append(inst)
            bb.instructions[:] = out


# ---------------------------------------------------------------- device kernel
BF16 = mybir.dt.bfloat16
F16 = mybir.dt.float16

# lmax/sqrt(q) ~ quadratic fit in (sd, s2) = (detR/q^1.5, C0/q^2); start point
# for the fixed-weight Newton iteration (lstsq on the actual feature
# distribution, rms resid 0.022).
B_QUAD = [1.429768871731997, 2.372311619434571, -0.22358362125748854,
          1.6180680207138314, -7.9808587338341646, -0.09846030115589527]


class Slots:
    """Workspace allocator: fresh pool tile per logical value, tag-recycled
    so SBUF footprint stays bounded at n slots x bufs."""

    def __init__(self, pool, n, shape, dtype, prefix):
        self.pool = pool
        self.shape = list(shape)
        self.dtype = dtype
        self.prefix = prefix
        self.free = list(range(n))[::-1]
        self.named = {}
        self.peak = 0

    def new(self, name):
        j = self.free.pop()
        t = self.pool.tile(
            self.shape, self.dtype, name=f"{self.prefix}{j}_{name}",
            tag=f"{self.prefix}{j}",
        )
        self.named[name] = (j, t)
        self.peak = max(self.peak, len(self.named))
        return t

    def __getitem__(self, name):
        return self.named[name][1]

    def drop(self, *names):
        for nm in names:
            j, _ = self.named.pop(nm)
            self.free.append(j)


DEBUG_TAPS = {}  # name -> dram tensor; filled by build_nc(debug=True)


def _tap(nc, name, ap, sb, tn):
    if DEBUG_TAPS and (sb, tn) == (0, 0) and name in DEBUG_TAPS:
        nc.sync.dma_start(out=DEBUG_TAPS[name][:], in_=ap)


def _emit_tile(nc, tc, pools, xm_s, xt_s, gm_s, gt_s, out_dram, sb, tn, pfx):
    """One [128, FD] output tile, all-fp16 elementwise pipeline; generator
    yielding at stage boundaries so two tiles can interleave their per-engine
    instruction streams (software pipelining).

    R rows land in PSUM (Xm pre-scaled by 1/32 so q, C0 stay in fp16 range);
    q, C0 = 2 tr(M^2) - q^2, detR computed in fp16 (Vector 2x mode);
    lmax via constant-damped Newton y <- y - w*p(y) on the normalized quartic
    y^4 - 2 y^2 + s1 y + s2, started from a quadratic fit y0(sd, s2).
    """
    psum_row, psum_n, wide, nb, nf, outp = pools
    V, G, SC = nc.vector, nc.gpsimd, nc.scalar

    ssl = slice(sb * 128, (sb + 1) * 128)
    tsl = slice(tn * FD, (tn + 1) * FD)

    W = Slots(wide, 8, [128, 3, FD], F16, f"W{pfx}")
    B = Slots(nb, 12, [128, FD], F16, f"B{pfx}")
    F = Slots(nf, 6, [128, FD], F32, f"F{pfx}")

    def wtt(dst, x, y, op, eng=V):
        eng.tensor_tensor(out=dst, in0=x, in1=y, op=op)

    # --- PE: R rows (R_kj = Xm_k . Xt_j over atoms) + N = Gm+Gt ------------
    # One PSUM buffer per parity: rows rotate through it; copy k frees it
    # for matmul k+1 while the other parity's matmuls keep PE busy.
    npl = psum_n.tile([128, FD], F32, name="npsum", tag=f"np{pfx}")
    nc.tensor.matmul(npl[:], gm_s[:, ssl], gt_s[:, tsl], start=True, stop=True)
    rows = []
    for k in range(3):
        pr = psum_row.tile([128, 3, FD], F32, name=f"pr{k}", tag=f"pr{pfx}")
        for j in range(3):
            nc.tensor.matmul(
                pr[:, j, :], xm_s[:, k, ssl], xt_s[:, j, tsl],
                start=True, stop=True,
            )
        rb = W.new(f"row{k}")
        SC.copy(rb[:], pr[:])
        rows.append(rb)
    yield

    def RP(k, j):  # fp16 R_kj plane
        return W[f"row{k}"][:, j, :]

    # --- M = R^T R diagonal -> q (V wide fp16) -----------------------------
    for k in range(3):
        sq = W.new(f"sq{k}")
        wtt(sq[:], W[f"row{k}"][:], W[f"row{k}"][:], ALU.mult, V)
    md1 = W.new("md1")
    wtt(md1[:], W["sq0"][:], W["sq1"][:], ALU.add, V)
    W.drop("sq0", "sq1")
    mdiag = W.new("mdiag")
    wtt(mdiag[:], md1[:], W["sq2"][:], ALU.add, V)
    W.drop("sq2", "md1")
    yield

    q1 = B.new("q1")
    wtt(q1[:], mdiag[:, 0, :], mdiag[:, 1, :], ALU.add, V)
    q16 = B.new("q16")
    wtt(q16[:], q1[:], mdiag[:, 2, :], ALU.add, V)
    B.drop("q1")
    sqMd = W.new("sqMd")
    wtt(sqMd[:], mdiag[:], mdiag[:], ALU.mult, V)
    W.drop("mdiag")
    z1 = B.new("z1")
    wtt(z1[:], sqMd[:, 0, :], sqMd[:, 1, :], ALU.add, G)
    z2 = B.new("z2")
    wtt(z2[:], z1[:], sqMd[:, 2, :], ALU.add, G)
    B.drop("z1")
    W.drop("sqMd")
    yield

    # --- M off-diagonal -> z4 (V wide fp16) --------------------------------
    for k in range(3):
        p = W.new(f"prod{k}")
        rb = W[f"row{k}"]
        wtt(p[:, 0:2, :], rb[:, 0:2, :], rb[:, 1:3, :], ALU.mult, V)
        wtt(p[:, 2, :], rb[:, 2, :], rb[:, 0, :], ALU.mult, V)
    mo1 = W.new("mo1")
    wtt(mo1[:], W["prod0"][:], W["prod1"][:], ALU.add, V)
    W.drop("prod0", "prod1")
    moff = W.new("moff")
    wtt(moff[:], mo1[:], W["prod2"][:], ALU.add, V)
    W.drop("prod2", "mo1")
    sqMo = W.new("sqMo")
    wtt(sqMo[:], moff[:], moff[:], ALU.mult, V)
    W.drop("moff")
    z3 = B.new("z3")
    wtt(z3[:], sqMo[:, 0, :], sqMo[:, 1, :], ALU.add, G)
    z4 = B.new("z4")
    wtt(z4[:], z3[:], sqMo[:, 2, :], ALU.add, G)
    B.drop("z3")
    W.drop("sqMo")
    yield

    # --- detR (fp16): det = a(ei-fh) - b(di-fg) + c(dh-eg) -----------------
    d_, e_, f_ = RP(1, 0), RP(1, 1), RP(1, 2)
    g_, h_, i_ = RP(2, 0), RP(2, 1), RP(2, 2)
    detA = W.new("detA")
    wtt(detA[:, 0, :], e_, i_, ALU.mult, G)
    wtt(detA[:, 1, :], d_, i_, ALU.mult, G)
    wtt(detA[:, 2, :], d_, h_, ALU.mult, G)
    detB = W.new("detB")
    wtt(detB[:, 0, :], f_, h_, ALU.mult, V)
    wtt(detB[:, 1, :], f_, g_, ALU.mult, V)
    wtt(detB[:, 2, :], e_, g_, ALU.mult, V)
    detC = W.new("detC")
    wtt(detC[:], detA[:], detB[:], ALU.subtract, V)
    W.drop("detA", "detB")
    tp = W.new("tp")
    wtt(tp[:], W["row0"][:], detC[:], ALU.mult, V)
    W.drop("detC", "row0", "row1", "row2")
    dt1 = B.new("dt1")
    wtt(dt1[:], tp[:, 0, :], tp[:, 1, :], ALU.subtract, G)
    detb = B.new("det")
    wtt(detb[:], dt1[:], tp[:, 2, :], ALU.add, G)
    B.drop("dt1")
    W.drop("tp")
    yield

    # --- C0 = 2*(2*z4 + z2) - q^2 (all fp16) -------------------------------
    qq = B.new("qq")
    wtt(qq[:], q16[:], q16[:], ALU.mult, V)
    u16 = B.new("u16")
    V.scalar_tensor_tensor(out=u16[:], in0=z4[:], scalar=2.0, in1=z2[:],
                           op0=ALU.mult, op1=ALU.add)
    B.drop("z2", "z4")
    C0b = B.new("C0")
    V.scalar_tensor_tensor(out=C0b[:], in0=u16[:], scalar=2.0, in1=qq[:],
                           op0=ALU.mult, op1=ALU.subtract)
    B.drop("u16", "qq")

    # --- features: sd = det*q^-1.5, s2 = C0*q^-2 via Ln/Exp (Scalar eng) ---
    sqq = F.new("sqq")
    SC.activation(sqq[:], q16[:], ACTF.Sqrt)
    lnq = F.new("lnq")
    SC.activation(lnq[:], q16[:], ACTF.Ln)
    B.drop("q16")
    rq15 = F.new("rq15")
    SC.activation(rq15[:], lnq[:], ACTF.Exp, scale=-1.5)
    rq2 = F.new("rq2")
    SC.activation(rq2[:], lnq[:], ACTF.Exp, scale=-2.0)
    F.drop("lnq")
    yield

    sdb = B.new("sd")
    wtt(sdb[:], detb[:], rq15[:], ALU.mult, G)
    B.drop("det")
    F.drop("rq15")
    s2b = B.new("s2")
    wtt(s2b[:], C0b[:], rq2[:], ALU.mult, G)
    B.drop("C0")
    F.drop("rq2")

    cb = B.new("c")
    V.tensor_scalar(out=cb[:], in0=sdb[:], scalar1=-8.0, scalar2=-2.0,
                    op0=ALU.mult, op1=ALU.add)  # c = s1 - 2 = -8*sd - 2

    # --- quadratic fit y0 = B0+B1*sd+B2*s2+B3*sd*s2+B4*sd^2+B5*s2^2 --------
    Q = B_QUAD
    sds2 = B.new("sds2")
    wtt(sds2[:], sdb[:], s2b[:], ALU.mult, V)
    sd2 = B.new("sd2")
    wtt(sd2[:], sdb[:], sdb[:], ALU.mult, V)
    s22 = B.new("s22")
    wtt(s22[:], s2b[:], s2b[:], ALU.mult, V)
    g1 = B.new("g1")
    V.tensor_scalar(out=g1[:], in0=sdb[:], scalar1=Q[1], scalar2=Q[0],
                    op0=ALU.mult, op1=ALU.add)
    g2 = B.new("g2")
    V.scalar_tensor_tensor(out=g2[:], in0=s2b[:], scalar=Q[2], in1=g1[:],
                           op0=ALU.mult, op1=ALU.add)
    B.drop("g1")
    g3 = B.new("g3")
    V.scalar_tensor_tensor(out=g3[:], in0=sds2[:], scalar=Q[3], in1=g2[:],
                           op0=ALU.mult, op1=ALU.add)
    B.drop("g2", "sds2")
    g4 = B.new("g4")
    V.scalar_tensor_tensor(out=g4[:], in0=sd2[:], scalar=Q[4], in1=g3[:],
                           op0=ALU.mult, op1=ALU.add)
    B.drop("g3", "sd2")
    y = B.new("y")
    V.scalar_tensor_tensor(out=y[:], in0=s22[:], scalar=Q[5], in1=g4[:],
                           op0=ALU.mult, op1=ALU.add)
    B.drop("g4", "s22")
    yield

    # --- damped Newton: y <- y - w*((y^2 + (s1-2))*y + s2) -----------------
    for _it in range(NEWTON_ITERS):
        y2 = B.new("y2")
        wtt(y2[:], y[:], y[:], ALU.mult, V)
        bb = B.new("b")
        wtt(bb[:], y2[:], cb[:], ALU.add, V)
        B.drop("y2")
        dd = B.new("d")
        wtt(dd[:], bb[:], y[:], ALU.mult, V)
        B.drop("b")
        num = B.new("num")
        wtt(num[:], dd[:], s2b[:], ALU.add, V)
        B.drop("d")
        yn = B.new("yn")
        V.scalar_tensor_tensor(out=yn[:], in0=num[:], scalar=-W_DAMP, in1=y[:],
                               op0=ALU.mult, op1=ALU.add)
        B.drop("y", "num")
        B.named["y"] = B.named.pop("yn")
        y = B["y"]
        yield
    B.drop("s2", "c")

    # --- output: rmsd = sqrt(relu((N - 64*lam)/(A+eps))) -------------------
    lam = F.new("lam")
    wtt(lam[:], y[:], sqq[:], ALU.mult, V)
    B.drop("y")
    F.drop("sqq")
    f2 = F.new("f2")
    V.scalar_tensor_tensor(out=f2[:], in0=lam[:], scalar=-64.0, in1=npl[:],
                           op0=ALU.mult, op1=ALU.add)
    F.drop("lam")
    rl = F.new("rl")
    V.tensor_scalar_max(rl[:], f2[:], 0.0)
    F.drop("f2")
    ot = outp.tile([128, FD], F32, name="out_t", tag=f"out{pfx}", bufs=2)
    SC.activation(ot[:], rl[:], ACTF.Sqrt, scale=1.0 / (A_ATOMS + EPS))
    F.drop("rl")
    nc.sync.dma_start(out=out_dram[ssl, tsl], in_=ot[:])


def build_nc(debug_taps=()):
    nc = bass.Bass()
    DEBUG_TAPS.clear()
    for nm in debug_taps:
        dt_ = BF16 if nm in ("q", "C0", "det", "sd", "s2") else F32
        DEBUG_TAPS[nm] = nc.declare_dram_parameter(
            f"dbg_{nm}", [128, FD], dt_, isOutput=True)
    xm = nc.declare_dram_parameter("xm", [A_ATOMS, 3, S_LOC], F32, isOutput=False)
    xt = nc.declare_dram_parameter("xt", [A_ATOMS, 3, T_FULL], F32, isOutput=False)
    gm = nc.declare_dram_parameter("gm", [2, S_LOC], F32, isOutput=False)
    gt = nc.declare_dram_parameter("gt", [2, T_FULL], F32, isOutput=False)
    out = nc.declare_dram_parameter("out", [S_LOC, T_FULL], F32, isOutput=True)

    with tile.TileContext(nc) as tc, nc.allow_low_precision(
        reason="bf16 coefficient pipeline; validated vs reference"
    ):
        with (
            tc.tile_pool(name="const", bufs=1) as const,
            tc.tile_pool(name="psum_row", bufs=1, space="PSUM") as psum_row,
            tc.tile_pool(name="psum_n", bufs=1, space="PSUM") as psum_n,
            tc.tile_pool(name="wide", bufs=1) as wide,
            tc.tile_pool(name="nb", bufs=1) as nb,
            tc.tile_pool(name="nf", bufs=1) as nf,
            tc.tile_pool(name="outp", bufs=1) as outp,
        ):
            xm_s = const.tile([A_ATOMS, 3, S_LOC], F32)
            xt_s = const.tile([A_ATOMS, 3, T_FULL], F32)
            gm_s = const.tile([2, S_LOC], F32)
            gt_s = const.tile([2, T_FULL], F32)
            nc.sync.dma_start(out=xm_s[:], in_=xm[:])
            for k in range(8):  # split the big replicated load across queues
                sl = slice(k * (T_FULL // 8), (k + 1) * (T_FULL // 8))
                nc.sync.dma_start(out=xt_s[:, :, sl], in_=xt[:, :, sl])
            nc.sync.dma_start(out=gm_s[:], in_=gm[:])
            nc.sync.dma_start(out=gt_s[:], in_=gt[:])

            pools = (psum_row, psum_n, wide, nb, nf, outp)
            tiles = [(sb, tn) for sb in range(S_LOC // 128)
                     for tn in range(T_FULL // FD)]
            for i in range(0, len(tiles), 2):
                gens = []
                for (sb, tn), pfx in zip(tiles[i:i + 2], "AB"):
                    gens.append(_emit_tile(nc, tc, pools, xm_s, xt_s, gm_s,
                                           gt_s, out, sb, tn, pfx))
                live = list(gens)
                while live:
                    for g in list(live):
                        try:
                            next(g)
                        except StopIteration:
                            live.remove(g)
    return nc


_NC_CACHE = None


def _get_nc():
    global _NC_CACHE
    if _NC_CACHE is None:
        nc = build_nc()
        _split_multi_waits(nc)
        _NC_CACHE = nc
    return _NC_CACHE


# ---------------------------------------------------------------- host wrapper
def kernel(X_mobile: np.ndarray, X_target: np.ndarray, **_ignored) -> np.ndarray:
    Xm = np.ascontiguousarray(X_mobile, dtype=np.float32)
    Xt = np.ascontiguousarray(X_target, dtype=np.float32)
    S, A, _ = Xm.shape
    T = Xt.shape[0]
    assert (S, A, T) == (S_FULL, A_ATOMS, T_FULL), (S, A, T)

    Xmc = Xm - Xm.mean(axis=1, keepdims=True)
    Xtc = Xt - Xt.mean(axis=1, keepdims=True)
    Gm = (Xmc * Xmc).sum(axis=(1, 2))
    Gt = (Xtc * Xtc).sum(axis=(1, 2))

    xt_r = np.ascontiguousarray(Xtc.transpose(1, 2, 0))  # [A, 3, T]
    gt2 = np.ascontiguousarray(
        np.stack([np.ones(T, np.float32), Gt.astype(np.float32)])
    )

    in_maps = []
    for c in range(N_CORES):
        sl = slice(c * S_LOC, (c + 1) * S_LOC)
        xm_l = np.ascontiguousarray(
            (Xmc[sl] * XM_SCALE).transpose(1, 2, 0)
        )  # [A, 3, S_loc]
        gm2 = np.ascontiguousarray(
            np.stack([Gm[sl].astype(np.float32), np.ones(S_LOC, np.float32)])
        )
        in_maps.append({"xm": xm_l, "xt": xt_r, "gm": gm2, "gt": gt2})

    nc = _get_nc()
    res = run_bass_kernel_spmd(nc, in_maps, list(range(N_CORES)))
    return np.concatenate([res.results[c]["out"] for c in range(N_CORES)], axis=0)


def run_traced(X_mobile, X_target):
    """test.py helper: same as kernel() but with NTFF tracing enabled."""
    Xm = np.ascontiguousarray(X_mobile, dtype=np.float32)
    Xt = np.ascontiguousarray(X_target, dtype=np.float32)
    Xmc = Xm - Xm.mean(axis=1, keepdims=True)
    Xtc = Xt - Xt.mean(axis=1, keepdims=True)
    Gm = (Xmc * Xmc).sum(axis=(1, 2))
    Gt = (Xtc * Xtc).sum(axis=(1, 2))
    xt_r = np.ascontiguousarray(Xtc.transpose(1, 2, 0))
    gt2 = np.ascontiguousarray(
        np.stack([np.ones(T_FULL, np.float32), Gt.astype(np.float32)])
    )
    in_maps = []
    for c in range(N_CORES):
        sl = slice(c * S_LOC, (c + 1) * S_LOC)
        xm_l = np.ascontiguousarray((Xmc[sl] * XM_SCALE).transpose(1, 2, 0))
        gm2 = np.ascontiguousarray(
            np.stack([Gm[sl].astype(np.float32), np.ones(S_LOC, np.float32)])
        )
        in_maps.append({"xm": xm_l, "xt": xt_r, "gm": gm2, "gt": gt2})
    nc = _get_nc()
    res = run_bass_kernel_spmd(nc, in_maps, list(range(N_CORES)), trace=True)
    out = np.concatenate([res.results[c]["out"] for c in range(N_CORES)], axis=0)
    return out, res

